# revision 46
# baseline (speedup 1.0000x reference)
"""Multi-head attention (B=8, N=2048, dim=64, heads=8) on 8 Trainium2 cores.

Sharding: batch-parallel — one batch element per NeuronCore, weights
replicated, no collectives. Per-core flash-style attention, fully
SBUF-resident (no HBM intermediates).

Schedule highlights:
- Two independent icx query streams interleaved chunk-by-chunk so the
  S -> exp -> A@V round trip never stalls the PE (single-buffered st
  tile per stream + shared double-buffered za accumulators = 8 PSUM
  banks).
- Softmax exp split across engines: ~55%% exact exp on ScalarE, ~45%%
  via a one-instruction Schraudolph approximation on VectorE (affine
  with int16 convert-on-write whose bits ARE the bf16 exp; softmax
  ratio cancels the sawtooth bias).
- The softmax denominator rides the A@V matmul as a leading ones
  column of V (PSUM partition 0), normalized with the fast custom-DVE
  reciprocal + gpsimd broadcast, z scaled straight out of PSUM.
- Tokens processed in a permuted order so the x load runs at 4KB per
  DMA descriptor; the output DMA applies the inverse permutation.
"""
import sys

import numpy as np


def _ensure_path():
    try:
        import concourse  # noqa: F401
    except ImportError:
        for p in (
            "/opt/trn_rl_repo",
            "/root/.axon_site",
            "/root/.axon_site/_ro/trn_rl_repo",
            "/root/.axon_site/_ro/pypackages",
        ):
            if p not in sys.path:
                sys.path.append(p)


_ensure_path()

import concourse.bacc as bacc  # noqa: E402
import concourse.mybir as mybir  # noqa: E402
import concourse.tile as tile  # noqa: E402
from concourse.bass_utils import run_bass_kernel_spmd  # noqa: E402
from concourse.masks import make_identity  # noqa: E402

B, N, D, H = 8, 2048, 64, 8
P = 128
NT = N // P          # 16 n-tiles of 128
IC = N // 512        # 4 query chunks of 512
SCALE = float(D) ** -0.5
F32 = mybir.dt.float32
F32R = mybir.dt.float32r
BF16 = mybir.dt.bfloat16
# Schraudolph bf16 exp on DVE: int16(x*EXP_A + EXP_C) bit-viewed as
# bf16 equals exp(x) to within ~3%; the softmax ratio cancels most of
# it. EXP_C = bf16 exponent bias (127<<7) minus a centering shift that
# zeroes the mean relative error so ACT-exact and DVE-approx key tiles
# mix without bias. The int16 convert-on-write does the rounding.
EXP_A = 128.0 / float(np.log(2.0))
EXP_C = 16256.0 - 6.55
I16 = mybir.dt.int16


def build_program(n_cores=B):
    nc = bacc.Bacc("TRN2", target_bir_lowering=False, debug=False,
                   num_devices=n_cores)
    x_d = nc.dram_tensor("x", [N, D], F32, kind="ExternalInput")
    wqkv_d = nc.dram_tensor("w_qkv", [D, 3 * H * D], F32, kind="ExternalInput")
    wout_d = nc.dram_tensor("w_out", [H * D, D], F32, kind="ExternalInput")
    bout_d = nc.dram_tensor("b_out", [D], F32, kind="ExternalInput")
    out_d = nc.dram_tensor("out", [N, D], F32, kind="ExternalOutput")

    with tile.TileContext(nc) as tc:
        with tc.tile_pool(name="const", bufs=1) as const:
            # x load first on every queue: everything downstream waits on it
            xall = const.tile([P, NT, D], F32, tag="xall")
            xr = x_d.ap().rearrange("(p t) d -> p t d", p=P)
            for qi, eng in enumerate((nc.sync, nc.gpsimd, nc.scalar,
                                      nc.sync)):
                eng.dma_start(xall[:, 4 * qi:4 * qi + 4, :],
                              xr[:, 4 * qi:4 * qi + 4, :])

            ident = const.tile([P, P], F32, tag="ident")
            make_identity(nc, ident[:])

            wsb = const.tile([D, 3 * H * D], F32R, tag="wqkv")
            nc.gpsimd.dma_start(wsb[:], wqkv_d.ap())
            wout_f32 = const.tile([P, 4, D], F32, tag="woutf")
            nc.gpsimd.dma_start(
                wout_f32[:], wout_d.ap().rearrange("(t p) d -> p t d", p=P))
            wout_sb = const.tile([P, 4, D], BF16, tag="wout")
            nc.vector.tensor_copy(wout_sb[:], wout_f32[:])
            b_row = const.tile([1, D], F32, tag="brow")
            nc.sync.dma_start(b_row[:], bout_d.ap().rearrange("(a d) -> a d", a=1))
            b_bc = const.tile([P, D], F32, tag="bbc")
            nc.gpsimd.partition_broadcast(b_bc[:], b_row[:])
            ones3 = const.tile([P, H, 1], F32, tag="ones3")
            nc.gpsimd.memset(ones3[:], 1.0)

            xT = const.tile([D, N], F32R, tag="xT")
            # qk_sb[0..3]: Q^T head-pairs [128, N]; qk_sb[4..7]: K^T pairs
            qk_sb = [const.tile([P, N], BF16, tag=f"qk{i}", name=f"qk{i}")
                     for i in range(8)]
            # V~ per n-tile: [128, H, 128]; per head: col 0 = ones (so the
            # softmax denominator lands at PSUM partition 0, a legal base
            # for the custom-DVE reciprocal + gpsimd broadcast), cols 1-63
            # zero, cols 64-127 = V (so z lands at partitions 64-127, a
            # legal 64-aligned base for the gpsimd normalize multiply).
            # The extra LDWEIGHTS columns hide under the 512-col A@V
            # streams via the PE load-ahead window.
            vt_sb = [const.tile([P, H, P], BF16, tag=f"vt{t}", name=f"vt{t}")
                     for t in range(NT)]
            zT = [const.tile([P, N], BF16, tag=f"zT{i}", name=f"zT{i}")
                  for i in range(4)]

            # ---- Phases 1+2 share one PSUM pool so setup tiles release
            # bank-by-bank into the attention pipeline (no phase barrier).
            # Budget: st 3 banks x 2 bufs + za0 + za1 = 8 banks; setup
            # borrows the za0/za1 slots, prefetch borrows st slots.
            with (
                tc.tile_pool(name="spsum", bufs=2,
                             space=bacc.bass.MemorySpace.PSUM) as spsum,
                tc.tile_pool(name="zpsum", bufs=1,
                             space=bacc.bass.MemorySpace.PSUM) as zpsum,
                tc.tile_pool(name="es", bufs=8) as es_pool,
                tc.tile_pool(name="sm", bufs=2) as sm_pool,
                tc.tile_pool(name="outp", bufs=3) as outp,
            ):
                # (tokens are processed in the permuted order pi(p,t) =
                # row p*NT+t: each SBUF partition loads one contiguous 4KB
                # block of x. Attention is permutation-equivariant over
                # tokens; the output DMA applies the inverse permutation.)
                alt = [0]

                def setup_psum():
                    # borrow the za0/za1 single-bank slots for setup matmuls
                    alt[0] ^= 1
                    return zpsum.tile([P, 512], F32, tag=f"za{alt[0]}",
                                      name="mps", bufs=2)

                def emit_qk(ct, icxs, pool_fn):
                    w_sl = wsb[:, ct * P:(ct + 1) * P]
                    for icx in icxs:
                        mp = pool_fn()
                        nc.tensor.matmul(
                            mp[0:P, 0:512], w_sl,
                            xT[:, icx * 512:(icx + 1) * 512],
                            start=True, stop=True)
                        nc.scalar.copy(
                            qk_sb[ct][:, icx * 512:(icx + 1) * 512],
                            mp[0:P, 0:512])

                def emit_vproj(t):
                    nc.vector.memset(vt_sb[t][:, :, 1:64], 0.0)
                    mp = setup_psum()
                    nc.tensor.matmul(
                        mp[0:P, 0:512], xT[:, t * P:(t + 1) * P],
                        wsb[:, 2 * H * D:3 * H * D],
                        start=True, stop=True)
                    nc.vector.tensor_copy(vt_sb[t][:, :, 0:1], ones3[:])
                    nc.vector.tensor_copy(
                        vt_sb[t][:, :, 64:P],
                        mp[0:P, 0:512].rearrange("p (h d) -> p h d", h=H))

                # setup, ordered so pair 0's inputs (head-pair 0 Q/K slices
                # for icx 0/1 and the first few V tiles) come out of the
                # engine queues first
                for t in range(8):
                    pp = setup_psum()
                    nc.tensor.transpose(pp[0:D, 0:P], xall[:, t, :],
                                        ident[:])
                    nc.vector.tensor_copy(xT[:, t * P:(t + 1) * P],
                                          pp[0:D, 0:P])
                emit_qk(4, [0, 1], setup_psum)
                emit_qk(0, [0, 1], setup_psum)
                for t in range(0, 4):
                    emit_vproj(t)
                for t in range(8, NT):
                    pp = setup_psum()
                    nc.tensor.transpose(pp[0:D, 0:P], xall[:, t, :],
                                        ident[:])
                    nc.vector.tensor_copy(xT[:, t * P:(t + 1) * P],
                                          pp[0:D, 0:P])
                emit_qk(5, [0, 1], setup_psum)
                emit_qk(1, [0, 1], setup_psum)
                for t in range(4, 10):
                    emit_vproj(t)
                for ct in (6, 2, 7, 3):
                    emit_qk(ct, [0, 1], setup_psum)
                for t in range(10, NT):
                    emit_vproj(t)
                for ct in (4, 0, 5, 1, 6, 2, 7, 3):
                    emit_qk(ct, [2, 3], setup_psum)
                # Head-pair interleaving: heads 2k / 2k+1 live in
                # complementary partition halves, so their K=64 S-matmuls
                # use disjoint PE row-groups and execute concurrently.
                #
                # TWO independent icx streams are interleaved chunk by
                # chunk: while stream A's exp is in flight on ACT/DVE, the
                # PE runs stream B's matmuls, so the S->exp->A@V round-trip
                # latency never stalls the PE. PSUM budget: st0/st1
                # single-buffered [P,1024] (2x2 banks) + za0/za1 rings
                # shared by the two streams (2x2 banks) = 8 banks.
                slices = [(hh, j) for j in range(NT) for hh in (0, 1)]
                chunks = [slices[i:i + 2] for i in range(0, 2 * NT, 2)]

                def use_dve(sx, _si):
                    # ~15 of 32 exps per pair on DVE, alternating slots
                    return sx % 2 == 0 and sx not in (0, 30)

                norm_pend = []  # [(za_pair, hp, icx), ...]

                def flush_norm(nc, after_unit=None):
                    # den row is at partition 0 (ones col 0 of vt): a legal
                    # base for the custom-DVE recip and gpsimd broadcast;
                    # z at 64-127 reads straight from PSUM (no staging).
                    # All recips are issued first so the gpsimd broadcasts
                    # pipeline behind them instead of ping-ponging.
                    units = []
                    while norm_pend:
                        za_p, hp_p, icx_p = norm_pend.pop(0)
                        for hh in (0, 1):
                            units.append((za_p[hh], hp_p, icx_p, hh))
                    bcs = []
                    for za_u, hp_p, icx_p, hh in units:
                        rc = sm_pool.tile([1, 512], F32, tag="rc",
                                          name="rc")
                        nc.vector.reciprocal_approx_fast(
                            rc[:], za_u[0:1, :])
                        bc = sm_pool.tile([P, 512], F32, tag="bc",
                                          name="bc")
                        nc.gpsimd.partition_broadcast(bc[:], rc[:])
                        bcs.append(bc)
                    for ui, (za_u, hp_p, icx_p, hh) in enumerate(units):
                        nc.vector.tensor_mul(
                            zT[hp_p][hh * 64:hh * 64 + 64,
                                     icx_p * 512:(icx_p + 1) * 512],
                            za_u[64:P, :], bcs[ui][64:P, :])
                        if after_unit is not None:
                            after_unit(ui)

                alt2 = [0]

                def stx_psum():
                    # borrow the st0/st1 slots (free at pair boundaries) for
                    # Q/K prefetch and output-projection matmuls
                    alt2[0] ^= 1
                    return spsum.tile([P, 1024], F32, tag=f"st{alt2[0]}",
                                      name="pfs", bufs=1)

                def emit_oproj(tiles):
                    for t in tiles:
                        op = stx_psum()
                        for ct in range(4):
                            nc.tensor.matmul(
                                op[0:P, 0:D], zT[ct][:, t * P:(t + 1) * P],
                                wout_sb[:, ct, :],
                                start=(ct == 0), stop=(ct == 3),
                                skip_group_check=True)
                        ot = outp.tile([P, D], F32, tag="ot", name="ot")
                        nc.vector.tensor_add(ot[:], op[0:P, 0:D], b_bc[:])
                        # permuted output rows -> 256B descriptors; spread
                        # across queues so the writes overlap compute
                        eng = (nc.sync, nc.gpsimd, nc.scalar)[t % 3]
                        eng.dma_start(
                            out_d.ap().rearrange("(p t) d -> p t d",
                                                 p=P)[:, t, :],
                            ot[:])

                # pair p runs streams (hp, ic0) and (hp, ic0+1)
                pair_list = [(hp, ic0) for ic0 in (0, 2)
                             for hp in range(H // 2)]
                for pi, (hp, ic0) in enumerate(pair_list):
                    icxs = (ic0, ic0 + 1)
                    # oproj for icx (0,1) tiles: their norms flushed at pair
                    # 4, so emit BEFORE this pair's norm flush (whose DVE
                    # work would otherwise delay the oproj bias-adds);
                    # icx (2,3) tiles drain in/after the last pair.
                    if pi == 5:
                        emit_oproj(range(0, 3))
                    elif pi == 6:
                        emit_oproj(range(3, 6))
                    elif pi == 7:
                        emit_oproj(range(6, 8))
                    flush_norm(nc)
                    qt = qk_sb[hp]
                    kt = qk_sb[4 + hp]
                    za = [[zpsum.tile([P, 512], F32, tag=f"za{hh}",
                                      name=f"za{hh}", bufs=2)
                           for hh in (0, 1)] for _ in (0, 1)]
                    jc = [[0, 0], [0, 0]]
                    pend = [[], []]

                    def flush_av(nc, si, lag=1, all_=False):
                        while pend[si] and (all_ or len(pend[si]) > lag):
                            rhs_p, chunk_p = pend[si].pop(0)
                            for ci, (hh, j) in enumerate(chunk_p):
                                nc.tensor.matmul(
                                    za[si][hh][:],
                                    vt_sb[j][:, 2 * hp + hh, :],
                                    rhs_p[ci],
                                    start=(jc[si][hh] == 0),
                                    stop=(jc[si][hh] == NT - 1),
                                    skip_group_check=True)
                                jc[si][hh] += 1

                    # Alternating slot plan with a slight A lead: A's
                    # chunks stay one ahead so A finishes 2 slots early,
                    # without ever placing same-stream chunks back to back
                    # (the single-buffered st needs a full slot of spacing).
                    slot_plan = [(0, 0)]
                    for k in range(15):
                        slot_plan += [(1, k), (0, k + 1)]
                    slot_plan += [(1, 15)]
                    done = [0, 0]
                    for sx, (si, cx) in enumerate(slot_plan):
                        chunk = chunks[cx]
                        # trailing A@Vs first: their es is ready, so the PE
                        # works through them while this slot's S matmuls
                        # wait out the st-slot turnaround (FIFO queue —
                        # emitting S first would head-of-line block them)
                        flush_av(nc, si, lag=3 if (si == 1 and
                                                   done[1] <= 4) else 2)
                        st = spsum.tile([P, 1024], F32, tag=f"st{si}",
                                        name=f"st{si}", bufs=1)
                        for ci, (hh, j) in enumerate(chunk):
                            r0 = hh * 64
                            nc.tensor.matmul(
                                st[:, ci * 512:(ci + 1) * 512],
                                kt[r0:r0 + 64, j * P:(j + 1) * P],
                                qt[r0:r0 + 64,
                                   icxs[si] * 512:(icxs[si] + 1) * 512],
                                start=True, stop=True)
                        if use_dve(sx, 0):
                            # Schraudolph bf16 exp on DVE: one affine with
                            # int16 convert-on-write; the bf16 bitcast view
                            # is contiguous, so the A@V matmuls stream at
                            # full rate.
                            es16 = es_pool.tile([P, 1024], I16,
                                                tag="es16", name="es16",
                                                bufs=8)
                            nc.vector.tensor_scalar(
                                es16[:], st[:], EXP_A * SCALE, EXP_C,
                                mybir.AluOpType.mult,
                                mybir.AluOpType.add)
                            ebf = es16[:].bitcast(BF16)
                            rhs = [ebf[:, ci * 512:(ci + 1) * 512]
                                   for ci in range(len(chunk))]
                        else:
                            es = es_pool.tile([P, 1024], BF16,
                                              tag="es", name="es",
                                              bufs=8)
                            nc.scalar.activation(
                                es[:], st[:],
                                mybir.ActivationFunctionType.Exp,
                                scale=SCALE)
                            rhs = [es[:, ci * 512:(ci + 1) * 512]
                                   for ci in range(len(chunk))]
                        pend[si].append((rhs, chunk))
                        done[si] += 1
                        if done[0] == len(chunks) and si == 0:
                            # stream A complete: drain its A@V pipeline and
                            # normalize now, freeing A's za slots mid-pair
                            flush_av(nc, 0, all_=True)
                            norm_pend.append((za[0], hp, icxs[0]))
                            flush_norm(nc)
                            if pi == 7:
                                # last pair: icx-2 tiles are fully
                                # normalized now, drain them early
                                emit_oproj(range(8, 12))
                    flush_av(nc, 1, all_=True)
                    norm_pend.append((za[1], hp, icxs[1]))
                def tail_oproj(ui):
                    if ui == 1:
                        emit_oproj(range(12, 14))

                flush_norm(nc, after_unit=tail_oproj)
                emit_oproj(range(14, 16))

    nc.compile()
    return nc


_PROG = None


def _get_program():
    global _PROG
    if _PROG is None:
        _PROG = build_program()
    return _PROG


def kernel(x, W_qkv, W_out, b_out):
    nc = _get_program()
    x = np.asarray(x, dtype=np.float32)
    wq = np.ascontiguousarray(np.asarray(W_qkv, dtype=np.float32))
    wo = np.ascontiguousarray(np.asarray(W_out, dtype=np.float32))
    bo = np.ascontiguousarray(np.asarray(b_out, dtype=np.float32))
    in_maps = [
        {"x": np.ascontiguousarray(x[i]), "w_qkv": wq, "w_out": wo,
         "b_out": bo}
        for i in range(B)
    ]
    res = run_bass_kernel_spmd(nc, in_maps, list(range(B)))
    return np.stack([res.results[i]["out"] for i in range(B)], axis=0)

